# revision 1
# baseline (speedup 1.0000x reference)
"""Causal self-attention (B=1, L=4096, E=1024, H=16) on 8 TRN2 NeuronCores.

Sharding: tensor-parallel over heads. Each core computes QKV + attention for
2 heads (all 4096 queries), then an AllToAll exchanges attention outputs so
each core holds all 1024 features for its 512-query slice, on which it runs
the output projection. Host concatenates the 8 row-slices.

Per-core layouts (feature-major / "transposed" throughout):
  xT      [E=1024, L=4096]   x transposed (host-side)
  wqkvT   [1024, 384]        cols: [q_h0|q_h1|k_h0|k_h1|v_h0|v_h1] (64 each)
  wprojT  [1024, 1024]       w_proj transposed (input-feature-major)
  out     [512, 1024]        this core's query-slice of the final output

On-chip: qT/kT stored [128, L] with head0 in partitions 0:63, head1 in
64:127 so row-tiled (K=64) score matmuls for both heads run concurrently.
Scores are computed transposed (sT[k, q] = k @ q^T) so softmax needs no
transposes: P = exp(sT/8) (no max subtraction needed -- scores are ~N(0,1)),
causal masking via a static upper-triangular mask multiply on the diagonal
128x128 sub-blocks, and y^T = [v|1]^T @ P gives both the weighted values and
the softmax denominators (psum row 64) in one accumulation chain.
"""

import numpy as np

import concourse.bacc as bacc
import concourse.mybir as mybir
from concourse import bass
from concourse.bass_utils import run_bass_kernel_spmd
from concourse.masks import make_identity, make_upper_triangular
from concourse.tile import TileContext

F32 = mybir.dt.float32
F32R = mybir.dt.float32r

NCORES = 8
H = 16            # total heads
HPC = 2           # heads per core
HD = 64           # head dim
E = 1024
L = 4096
LT = 512          # q-tile width (= max fp32 matmul moving dim / psum bank)
NQT = L // LT     # 8 q-tiles
KT = 128          # k-tile height (= max contraction)
NKT = L // KT     # 32 k-tiles
ECH = E // 128    # 8 e-chunks of 128 (qkv contraction)
LROWS = L // NCORES   # 512 output rows per core
SCALE = 1.0 / 8.0     # 1/sqrt(HD)


def r32(ap):
    """View an AP as float32r so the PE runs full-rate single-pass fp32."""
    return ap.bitcast(F32R)


def build_program(single_core=False, repeat=None, no_cc=False):
    # single_core=True replaces the AllToAll with a local DRAM copy so the
    # per-core program can run under the single-core cost-model timeline sim.
    # repeat=N wraps the computation in an on-device For loop so wall-clock
    # timing can resolve per-iteration kernel time above dispatch overhead.
    nc = bacc.Bacc(
        trn_type="TRN2",
        target_bir_lowering=False,
        debug=False,
        num_devices=1 if single_core else NCORES,
    )

    xT = nc.dram_tensor("xT", [E, L], F32, kind="ExternalInput")
    wqkvT = nc.dram_tensor("wqkvT", [E, 6 * HD], F32, kind="ExternalInput")
    wprojT = nc.dram_tensor("wprojT", [E, E], F32, kind="ExternalInput")
    out = nc.dram_tensor("out", [LROWS, E], F32, kind="ExternalOutput")

    # AllToAll bounce buffers: slice s of a2a_in is this core's normalized
    # y^T[:, s*512:(s+1)*512]; after the exchange, a2a_out[c] is core c's
    # contribution for OUR query slice -> stacked = [1024 features, 512].
    # half-major so each half-width exchange is a contiguous region
    HLT = LT // 2
    a2a_in = nc.dram_tensor("a2a_in", [2, NCORES, HPC * HD, HLT], F32)
    a2a_out = nc.dram_tensor("a2a_out", [2, NCORES, HPC * HD, HLT], F32)
    # bounce for per-query reciprocal softmax sums (partition-replicate src)
    rs_dram = nc.dram_tensor("rs_dram", [HPC * NQT, LT], F32)

    xT_r = xT[:].rearrange("(c p) l -> p c l", p=128)       # [128, 8, L]
    wqkvT_r = wqkvT[:].rearrange("(c p) w -> p c w", p=128)  # [128, 8, 384]
    wprojT_r = wprojT[:].rearrange("(c p) o -> p c o", p=128)  # [128, 8, 1024]
    a2a_out_r = a2a_out[:].rearrange("s c p l -> s p c l")   # [2, 128, 8, 256]

    with TileContext(nc) as tc:
        with (
            tc.tile_pool(name="const", bufs=1) as const_pool,
            tc.tile_pool(name="store", bufs=1) as store_pool,
            tc.tile_pool(name="xt", bufs=2) as xt_pool,
            tc.tile_pool(name="p2", bufs=3) as p2_pool,
            tc.tile_pool(name="misc", bufs=2) as misc_pool,
            tc.tile_pool(name="ps", bufs=1, space="PSUM") as ps_pool,
            tc.tile_pool(name="psy", bufs=2, space="PSUM") as psy_pool,
        ):
            # ---- constants / weights -------------------------------------
            ident = const_pool.tile([128, 128], F32)
            make_identity(nc, ident[:])
            # tri[p, j] = 1 if j >= p else 0: the causal mask of every
            # diagonal [128k x 128q] sub-block (keys on partitions).
            tri = const_pool.tile([128, 128], F32)
            make_upper_triangular(nc, tri[:], val=1.0, diag=True)

            # wqkv chunk DMAs are interleaved with the first x tile's chunks
            # (inside lt_units(0)) so the first qkv matmul starts ~2us in.
            wqkv_sb = const_pool.tile([128, ECH, 6 * HD], F32)
            # wproj is only needed in phase 3; its 4MB DMA is emitted later
            # so it doesn't delay the first xT tiles.
            wproj_sb = const_pool.tile([128, ECH, E], F32)
            if repeat is not None:
                nc.sync.dma_start(out=r32(wproj_sb[:]), in_=r32(wprojT_r))

            # persistent activations
            qT_sb = store_pool.tile([128, L], F32)   # [q_h0;q_h1] feature-major
            kT_sb = store_pool.tile([128, L], F32)
            vT_sb = store_pool.tile([128, L], F32, tag="vT_sb")
            # v in key-major layout + ones column for softmax denominators.
            # (memset cannot write f32r directly -- ISA check -- so memset a
            # plain f32 tile and tensor_copy it, which can.)
            v_sb = store_pool.tile([128, HPC, NKT, HD + 1], F32)
            ones_src = const_pool.tile([128, HPC * NKT], F32)
            nc.vector.memset(ones_src[:], 1.0)
            nc.vector.tensor_copy(
                out=r32(v_sb[:, :, :, HD:HD + 1].rearrange("p a b c -> p (a b c)")),
                in_=ones_src[:],
            )

            dests = [qT_sb, kT_sb, vT_sb]

            def emit_all():
                # -- unit-granular emitters so qkv work interleaves between
                # attention k-tiles --
                def lt_units(lt):
                    xt = xt_pool.tile([128, ECH, LT], F32, name=f"xt{lt}",
                                      tag="xt")

                    def dma_unit():
                        for ec in range(ECH):
                            if lt == 0:
                                nc.sync.dma_start(out=r32(wqkv_sb[:, ec, :]),
                                                  in_=r32(wqkvT_r[:, ec, :]))
                            nc.sync.dma_start(
                                out=r32(xt[:, ec, :]),
                                in_=r32(xT_r[:, ec, lt * LT:(lt + 1) * LT]))
                    yield dma_unit

                    def g_unit(g):
                        ps = ps_pool.tile([128, 2, LT], F32,
                                          name=f"qkv{lt}{g}", tag="qkv", bufs=1)
                        for ec in range(ECH):
                            nc.tensor.matmul(
                                ps[:, 0, :],
                                lhsT=r32(wqkv_sb[:, ec, g * 128:(g + 1) * 128]),
                                rhs=r32(xt[:, ec, :]),
                                start=(ec == 0),
                                stop=(ec == ECH - 1),
                            )
                        nc.vector.tensor_copy(
                            out=r32(dests[g][:, lt * LT:(lt + 1) * LT]),
                            in_=ps[:, 0, :])
                    for g in range(3):
                        yield (lambda g=g: g_unit(g))

                    def tp_unit(h, j):
                        kt = lt * (LT // KT) + j
                        tp = ps_pool.tile([128, 2, LT], F32,
                                          name=f"tp{kt}{h}", tag="qkv", bufs=1)
                        nc.tensor.transpose(
                            tp[:, 0, 0:HD],
                            in_=vT_sb[h * HD:(h + 1) * HD, kt * KT:(kt + 1) * KT],
                            identity=ident[h * HD:(h + 1) * HD,
                                           h * HD:(h + 1) * HD],
                        )
                        nc.vector.tensor_copy(
                            out=r32(v_sb[:, h, kt, 0:HD]), in_=tp[:, 0, 0:HD])
                    for h in range(HPC):
                        for j in range(LT // KT):
                            yield (lambda h=h, j=j: tp_unit(h, j))

                def emit_scores(qt, kt):
                    # first useful query column of this k-tile
                    c0 = max(0, kt * KT - qt * LT)
                    s2 = ps_pool.tile([128, 2, LT], F32,
                                      name=f"s{qt}_{kt}", tag="s", bufs=2)
                    for h in range(HPC):
                        nc.tensor.matmul(
                            s2[:, h, c0:LT],
                            lhsT=r32(kT_sb[h * HD:(h + 1) * HD,
                                           kt * KT:(kt + 1) * KT]),
                            rhs=r32(qT_sb[h * HD:(h + 1) * HD,
                                          qt * LT + c0:(qt + 1) * LT]),
                            start=True,
                            stop=True,
                        )
                    return s2

                def emit_kt(qt, kt, first, last, yps, s2):
                    # PE is FIFO: av(kt) waits on exp(kt), so scores(kt+1)
                    # must already be in the queue BEFORE av(kt) (the caller
                    # emits scores one k-tile ahead) or the whole per-kt
                    # chain serializes at exp+av latency instead of exp rate.
                    c0 = max(0, kt * KT - qt * LT)
                    p2 = p2_pool.tile([128, 2, LT], F32,
                                      name=f"p{qt}_{kt}", tag="p2")
                    nc.scalar.activation(
                        out=r32(p2[:, :, c0:LT]),
                        in_=s2[:, :, c0:LT],
                        func=mybir.ActivationFunctionType.Exp,
                        scale=SCALE,
                    )
                    if kt * KT >= qt * LT:  # diagonal band: causal mask
                        # partial region is exactly columns [c0, c0+128);
                        # beyond it fully allowed, below c0 not computed.
                        for h in range(HPC):
                            nc.vector.tensor_mul(
                                r32(p2[:, h, c0:c0 + KT]),
                                p2[:, h, c0:c0 + KT],
                                tri[:, :],
                            )
                    for h in range(HPC):
                        nc.tensor.matmul(
                            yps[h][0:HD + 1, c0:LT],
                            lhsT=r32(v_sb[:, h, kt, 0:HD + 1]),
                            rhs=r32(p2[:, h, c0:LT]),
                            start=(kt == first),
                            stop=(kt == last),
                        )

                def emit_qt_tail(qt, yps):
                    # normalize y = y_unnorm / sums (sums = psum row 64);
                    # evacuate psum first so the y banks free immediately
                    for h in range(HPC):
                        row = qt * HPC + h
                        ysb = misc_pool.tile([HD + 1, LT], F32,
                                             name=f"ysb{row}", tag="ysb")
                        nc.vector.tensor_copy(out=ysb[:], in_=yps[h][:])
                        rec = misc_pool.tile([HD + 1, LT], F32,
                                             name=f"rec{row}", tag="rec")
                        nc.vector.reciprocal(out=rec[HD:HD + 1, :],
                                             in_=ysb[HD:HD + 1, :])
                        nc.sync.dma_start(out=rs_dram[row, :],
                                          in_=rec[HD:HD + 1, :])
                        rb = misc_pool.tile([HD, LT], F32,
                                            name=f"rb{row}", tag="rb")
                        nc.sync.dma_start(
                            out=rb[:],
                            in_=rs_dram[row:row + 1, :].broadcast_to([HD, LT]))
                        yn = misc_pool.tile([HD, LT], F32,
                                            name=f"yn{row}", tag="yn")
                        nc.vector.tensor_mul(yn[:], ysb[0:HD, :], rb[:])
                        for s in range(2):
                            nc.sync.dma_start(
                                out=a2a_in[s, qt, h * HD:(h + 1) * HD, :],
                                in_=yn[:, s * HLT:(s + 1) * HLT])

                # Schedule: attention q-tile i right after its own l-tile
                # (it needs qT of l-tile i and kT/v of l-tiles <= i). Exp is
                # the bottleneck; cumulative exp after the first unlock far
                # exceeds the remaining qkv chain, so this is ACT-bound.
                def attn_group(qt):
                    nkt = 4 * qt + 4
                    yps = [psy_pool.tile([HD + 1, LT], F32, tag="y",
                                         name=f"yps{qt}_{h}")
                           for h in range(HPC)]
                    s_next = emit_scores(qt, 0)
                    for kt in range(nkt):
                        s_cur = s_next
                        if kt + 1 < nkt:
                            s_next = emit_scores(qt, kt + 1)
                        emit_kt(qt, kt, 0, nkt - 1, yps, s_cur)
                    emit_qt_tail(qt, yps)

                def seq_lt(lt):
                    for u in lt_units(lt):
                        u()

                seq_lt(0)
                attn_group(0)
                seq_lt(1)
                if repeat is None:
                    nc.sync.dma_start(out=r32(wproj_sb[:]), in_=r32(wprojT_r))
                attn_group(1)
                for i in range(2, NQT):
                    seq_lt(i)
                    attn_group(i)

                # ---- phase 3: exchange + output projection ---------------
                # Two half-width AllToAlls; the projection row-tiles whose
                # ya columns live in a half are emitted right after it so
                # they overlap the second exchange.
                ya_sb = store_pool.tile([128, ECH, LT], F32, name="ya_sb",
                                        tag="vT_sb")

                def emit_proj(rt):
                    ps = ps_pool.tile([128, 2, LT], F32, tag="qkv", bufs=1,
                                      name=f"proj{rt}")
                    for fc in range(ECH):
                        for nt in range(E // LT):
                            nc.tensor.matmul(
                                ps[:, nt, :],
                                lhsT=r32(ya_sb[:, fc, rt * 128:(rt + 1) * 128]),
                                rhs=r32(wproj_sb[:, fc, nt * LT:(nt + 1) * LT]),
                                start=(fc == 0),
                                stop=(fc == ECH - 1),
                            )
                    for nt in range(E // LT):
                        o_sb = misc_pool.tile([128, LT], F32, tag="o",
                                              name=f"o{rt}{nt}")
                        nc.scalar.copy(out=o_sb[:], in_=ps[:, nt, :])
                        nc.sync.dma_start(
                            out=out[rt * 128:(rt + 1) * 128,
                                    nt * LT:(nt + 1) * LT],
                            in_=o_sb[:],
                        )

                for half in range(2):
                    if single_core or no_cc:
                        nc.sync.dma_start(out=a2a_out[half], in_=a2a_in[half])
                    else:
                        nc.gpsimd.collective_compute(
                            "AllToAll",
                            mybir.AluOpType.bypass,
                            replica_groups=[list(range(NCORES))],
                            ins=[a2a_in[half]],
                            outs=[a2a_out[half]],
                        )
                    for fc in range(ECH):
                        nc.sync.dma_start(
                            out=r32(ya_sb[:, fc, half * HLT:(half + 1) * HLT]),
                            in_=r32(a2a_out_r[half, :, fc, :]))
                    emit_proj(2 * half)
                    emit_proj(2 * half + 1)

            if repeat is not None:
                with tc.For_i(0, repeat, 1):
                    emit_all()
            else:
                emit_all()

    nc.compile()
    return nc


def shard_inputs(x, w_attn, w_proj):
    """Host-side prep: transpose + per-core head shards."""
    x = np.asarray(x, dtype=np.float32)
    w_attn = np.asarray(w_attn, dtype=np.float32)
    w_proj = np.asarray(w_proj, dtype=np.float32)
    xT = np.ascontiguousarray(x.reshape(L, E).T)          # [E, L]
    wprojT = np.ascontiguousarray(w_proj.T)               # [E, E]
    wq, wk, wv = w_attn[0:E], w_attn[E:2 * E], w_attn[2 * E:3 * E]
    in_maps = []
    for c in range(NCORES):
        h0, h1 = HPC * c, HPC * c + 1
        cols = np.concatenate([
            wq[h0 * HD:(h0 + 1) * HD], wq[h1 * HD:(h1 + 1) * HD],
            wk[h0 * HD:(h0 + 1) * HD], wk[h1 * HD:(h1 + 1) * HD],
            wv[h0 * HD:(h0 + 1) * HD], wv[h1 * HD:(h1 + 1) * HD],
        ], axis=0)                                         # [384, E]
        in_maps.append({
            "xT": xT,
            "wqkvT": np.ascontiguousarray(cols.T),         # [E, 384]
            "wprojT": wprojT,
        })
    return in_maps


_NC_CACHE = None


def get_program():
    global _NC_CACHE
    if _NC_CACHE is None:
        _NC_CACHE = build_program()
    return _NC_CACHE


def kernel(x, w_attn, w_proj):
    nc = get_program()
    in_maps = shard_inputs(x, w_attn, w_proj)
    res = run_bass_kernel_spmd(nc, in_maps, list(range(NCORES)))
    out = np.concatenate([res.results[c]["out"] for c in range(NCORES)], axis=0)
    return out.reshape(1, L, E).astype(np.float32)



# revision 11
# speedup vs baseline: 1.0511x; 1.0511x over previous
"""Causal self-attention (B=1, L=4096, E=1024, H=16) on 8 TRN2 NeuronCores.

Sharding: tensor-parallel over heads. Each core computes QKV + attention for
2 heads (all 4096 queries) in bf16 (tolerance 2e-2 >> bf16 rounding), then
query-slices of the normalized attention output are exchanged so each core
holds all 1024 features for the 512 queries it owns, on which it runs the
output projection. Core c owns queries [c*256,(c+1)*256) of each 2048-query
half, so the first exchange fires halfway through attention and its
projection overlaps the remaining attention groups.

Per-core layouts (feature-major / "transposed" throughout):
  xT      [E=1024, L=4096]   x transposed (host-side), bf16
  wqkvT   [1024, 384]        cols: [q_h0|q_h1|k_h0|k_h1|v_h0|v_h1] (64 each)
  wprojT  [1024, 1024]       w_proj transposed (input-feature-major)
  out     [2, 256, 1024]     per owned query-slice of the final output

On-chip: qT/kT stored [128, L] bf16 with head0 in partitions 0:63, head1 in
64:127 so row-tiled (K=64) score matmuls for both heads run concurrently.
Scores are computed transposed (sT[k, q] = k @ q^T) so softmax needs no
transposes: P = exp(sT/8) (no max subtraction needed -- scores are ~N(0,1)),
causal masking via a static upper-triangular mask multiply on the diagonal
128x128 sub-blocks, and y^T = [v|1]^T @ P gives both the weighted values and
the softmax denominators (psum row 64) in one accumulation chain. The
per-query reciprocal denominators are replicated across partitions with a
1-contraction PE matmul against a ones row (no DRAM bounce).

Scheduling: qkv work for l-tile i+1 is interleaved between attention k-tiles
of q-tile i so the scalar engine (exp, the co-bottleneck) never drains while
the PE runs the qkv chain.
"""

import numpy as np
import ml_dtypes

import concourse.bacc as bacc
import concourse.mybir as mybir
from concourse import bass
from concourse.bass_utils import run_bass_kernel_spmd
from concourse.masks import make_identity, make_upper_triangular
from concourse.tile import TileContext

F32 = mybir.dt.float32
BF16 = mybir.dt.bfloat16

NCORES = 8
H = 16            # total heads
HPC = 2           # heads per core
HD = 64           # head dim
E = 1024
L = 4096
LT = 512          # q-tile width (= max fp32 matmul moving dim / psum bank)
NQT = L // LT     # 8 q-tiles
KT = 128          # k-tile height (= max contraction)
NKT = L // KT     # 32 k-tiles
ECH = E // 128    # 8 e-chunks of 128 (qkv contraction)
QOWN = 256        # queries owned per core per L/2 half
SCALE = 1.0 / 8.0  # 1/sqrt(HD)


def build_program(single_core=False, repeat=None, no_cc=False):
    # single_core=True replaces the AllToAll with a local DRAM copy so the
    # per-core program can run under the single-core cost-model timeline sim.
    # repeat=N wraps the computation in an on-device For loop so wall-clock
    # timing can resolve per-iteration kernel time above dispatch overhead.
    nc = bacc.Bacc(
        trn_type="TRN2",
        target_bir_lowering=False,
        debug=False,
        num_devices=1 if single_core else NCORES,
    )

    xT = nc.dram_tensor("xT", [E, L], BF16, kind="ExternalInput")
    wqkvT = nc.dram_tensor("wqkvT", [E, 6 * HD], BF16, kind="ExternalInput")
    wprojT = nc.dram_tensor("wprojT", [E, E], BF16, kind="ExternalInput")
    out = nc.dram_tensor("out", [2, QOWN, E], BF16, kind="ExternalOutput")

    # Exchange buffers: a2a_in[g, d] is this core's feature band of the
    # normalized y^T for the 256 queries of half g owned by core d; after the
    # exchange, a2a_out[g, s] is core s's band for OUR queries of half g.
    a2a_in = nc.dram_tensor("a2a_in", [2, NCORES, HPC * HD, QOWN], BF16)
    a2a_out = nc.dram_tensor("a2a_out", [2, NCORES, HPC * HD, QOWN], BF16)

    xT_r = xT[:].rearrange("(c p) l -> p c l", p=128)        # [128, 8, L]
    wqkvT_r = wqkvT[:].rearrange("(c p) w -> p c w", p=128)  # [128, 8, 384]
    wprojT_r = wprojT[:].rearrange("(c p) o -> p c o", p=128)  # [128, 8, 1024]
    a2a_out_r = a2a_out[:].rearrange("g s p q -> g p s q")   # [2, 128, 8, 256]

    with TileContext(nc) as tc:
        with (
            nc.allow_low_precision(reason="2e-2 tolerance; bf16 throughout"),
            tc.tile_pool(name="const", bufs=1) as const_pool,
            tc.tile_pool(name="store", bufs=1) as store_pool,
            tc.tile_pool(name="xt", bufs=2) as xt_pool,
            tc.tile_pool(name="p2", bufs=3) as p2_pool,
            tc.tile_pool(name="misc", bufs=2) as misc_pool,
            tc.tile_pool(name="ps", bufs=1, space="PSUM") as ps_pool,
            tc.tile_pool(name="psy", bufs=2, space="PSUM") as psy_pool,
        ):
            # ---- constants / weights -------------------------------------
            ident = const_pool.tile([128, 128], BF16)
            make_identity(nc, ident[:])
            # tri[p, j] = 1 if j >= p else 0: the causal mask of every
            # diagonal [128k x 128q] sub-block (keys on partitions).
            tri = const_pool.tile([128, 128], BF16)
            make_upper_triangular(nc, tri[:], val=1.0, diag=True)
            ones_row = const_pool.tile([1, HD], BF16)
            nc.vector.memset(ones_row[:], 1.0)

            wqkv_sb = const_pool.tile([128, ECH, 6 * HD], BF16)
            # wproj is only needed once projections start; its DMA is emitted
            # after the first l-tile so it doesn't delay the pipeline ramp.
            wproj_sb = const_pool.tile([128, ECH, E], BF16)
            if repeat is not None:
                nc.sync.dma_start(out=wproj_sb[:], in_=wprojT_r)

            # persistent activations
            qT_sb = store_pool.tile([128, L], BF16)  # [q_h0;q_h1] feature-major
            kT_sb = store_pool.tile([128, L], BF16)
            vT_sb = store_pool.tile([128, L], BF16, tag="vT_sb")
            # v in key-major layout + ones column for softmax denominators
            v_sb = store_pool.tile([128, HPC, NKT, HD + 1], BF16)
            ones_src = const_pool.tile([128, HPC * NKT], BF16)
            nc.vector.memset(ones_src[:], 1.0)
            nc.vector.tensor_copy(
                out=v_sb[:, :, :, HD:HD + 1].rearrange("p a b c -> p (a b c)"),
                in_=ones_src[:],
            )
            # post-exchange activations: all 1024 features of owned queries
            ya_sb = store_pool.tile([128, 2, ECH, QOWN], BF16, tag="vT_sb",
                                    name="ya_sb")

            dests = [qT_sb, kT_sb, vT_sb]

            def emit_all():
                # -- unit-granular emitters; units of l-tile lt+1 are
                # interleaved between attention k-tiles of q-tile lt --
                def lt_units(lt):
                    xt = xt_pool.tile([128, ECH, LT], BF16, name=f"xt{lt}",
                                      tag="xt")
                    EH = ECH // 2

                    def dma_a():
                        if lt == 0:
                            nc.sync.dma_start(out=wqkv_sb[:, 0:EH, :],
                                              in_=wqkvT_r[:, 0:EH, :])
                        nc.sync.dma_start(
                            out=xt[:, 0:EH, :],
                            in_=xT_r[:, 0:EH, lt * LT:(lt + 1) * LT])
                    yield dma_a

                    def dma_b():
                        if lt == 0:
                            nc.sync.dma_start(out=wqkv_sb[:, EH:ECH, :],
                                              in_=wqkvT_r[:, EH:ECH, :])
                        nc.sync.dma_start(
                            out=xt[:, EH:ECH, :],
                            in_=xT_r[:, EH:ECH, lt * LT:(lt + 1) * LT])
                    yield dma_b

                    # qkv matmul chains, split in halves so each interleave
                    # slot costs < one exp tile and the ACT queue never drains
                    for g in range(3):
                        ps = ps_pool.tile([128, LT], F32,
                                          name=f"qkv{lt}{g}", tag="qkv",
                                          bufs=1)

                        def g_half(g, ps, e0, e1):
                            for ec in range(e0, e1):
                                nc.tensor.matmul(
                                    ps[:],
                                    lhsT=wqkv_sb[:, ec, g * 128:(g + 1) * 128],
                                    rhs=xt[:, ec, :],
                                    start=(ec == 0),
                                    stop=(ec == ECH - 1),
                                )
                            if e1 == ECH:
                                nc.vector.tensor_copy(
                                    out=dests[g][:, lt * LT:(lt + 1) * LT],
                                    in_=ps[:])
                        yield (lambda g=g, ps=ps: g_half(g, ps, 0, EH))
                        yield (lambda g=g, ps=ps: g_half(g, ps, EH, ECH))

                    def tp_unit(h, j):
                        kt = lt * (LT // KT) + j
                        tp = ps_pool.tile([128, LT], BF16,
                                          name=f"tp{kt}{h}", tag="qkv", bufs=1)
                        nc.tensor.transpose(
                            tp[:, 0:HD],
                            in_=vT_sb[h * HD:(h + 1) * HD,
                                      kt * KT:(kt + 1) * KT],
                            identity=ident[h * HD:(h + 1) * HD,
                                           h * HD:(h + 1) * HD],
                        )
                        nc.vector.tensor_copy(
                            out=v_sb[:, h, kt, 0:HD], in_=tp[:, 0:HD])
                    for h in range(HPC):
                        for j in range(LT // KT):
                            yield (lambda h=h, j=j: tp_unit(h, j))

                def emit_scores(qt, kt):
                    # first useful query column of this k-tile
                    c0 = max(0, kt * KT - qt * LT)
                    s2 = ps_pool.tile([128, 2, LT], F32,
                                      name=f"s{qt}_{kt}", tag="s", bufs=2)
                    for h in range(HPC):
                        nc.tensor.matmul(
                            s2[:, h, c0:LT],
                            lhsT=kT_sb[h * HD:(h + 1) * HD,
                                       kt * KT:(kt + 1) * KT],
                            rhs=qT_sb[h * HD:(h + 1) * HD,
                                      qt * LT + c0:(qt + 1) * LT],
                            start=True,
                            stop=True,
                        )
                    return s2

                def emit_kt(qt, kt, first, last, yps, s2):
                    # PE is FIFO: av(kt) waits on exp(kt), so scores(kt+1)
                    # must already be in the queue BEFORE av(kt) (the caller
                    # emits scores one k-tile ahead) or the whole per-kt
                    # chain serializes at exp+av latency instead of exp rate.
                    c0 = max(0, kt * KT - qt * LT)
                    p2 = p2_pool.tile([128, 2, LT], BF16,
                                      name=f"p{qt}_{kt}", tag="p2")
                    nc.scalar.activation(
                        out=p2[:, :, c0:LT],
                        in_=s2[:, :, c0:LT],
                        func=mybir.ActivationFunctionType.Exp,
                        scale=SCALE,
                    )
                    if kt * KT >= qt * LT:  # diagonal band: causal mask
                        for h in range(HPC):
                            nc.vector.tensor_mul(
                                p2[:, h, c0:c0 + KT],
                                p2[:, h, c0:c0 + KT],
                                tri[:, :],
                            )
                    for h in range(HPC):
                        nc.tensor.matmul(
                            yps[h][0:HD + 1, c0:LT],
                            lhsT=v_sb[:, h, kt, 0:HD + 1],
                            rhs=p2[:, h, c0:LT],
                            start=(kt == first),
                            stop=(kt == last),
                        )

                def emit_qt_tail(qt, yps):
                    # normalize y = y_unnorm / sums (sums = psum row 64) and
                    # stage the owned-query slices for the exchange
                    half, qq = divmod(qt, 4)
                    for h in range(HPC):
                        row = qt * HPC + h
                        ysb = misc_pool.tile([HD + 1, LT], BF16,
                                             name=f"ysb{row}", tag="ysb")
                        nc.vector.tensor_copy(out=ysb[:], in_=yps[h][:])
                        rec = misc_pool.tile([1, LT], BF16,
                                             name=f"rec{row}", tag="rec")
                        nc.vector.reciprocal(out=rec[:],
                                             in_=ysb[HD:HD + 1, :])
                        # replicate the per-query reciprocals across the 64
                        # value partitions with a K=1 matmul against ones
                        rb = psy_pool.tile([HD + 1, LT], F32, tag="y",
                                           name=f"rb{row}")
                        nc.tensor.matmul(rb[0:HD, :], lhsT=ones_row[0:1, :],
                                         rhs=rec[0:1, :], start=True,
                                         stop=True)
                        yn = misc_pool.tile([HD, LT], BF16,
                                            name=f"yn{row}", tag="yn")
                        nc.vector.tensor_mul(yn[:], ysb[0:HD, :],
                                             rb[0:HD, :])
                        for sub in range(2):
                            dst = 2 * qq + sub
                            nc.sync.dma_start(
                                out=a2a_in[half, dst, h * HD:(h + 1) * HD, :],
                                in_=yn[:, sub * QOWN:(sub + 1) * QOWN])

                def emit_exchange(half):
                    if single_core or no_cc:
                        nc.sync.dma_start(out=a2a_out[half],
                                          in_=a2a_in[half])
                    else:
                        nc.gpsimd.collective_compute(
                            "AllToAll",
                            mybir.AluOpType.bypass,
                            replica_groups=[list(range(NCORES))],
                            ins=[a2a_in[half]],
                            outs=[a2a_out[half]],
                        )
                    nc.sync.dma_start(out=ya_sb[:, half, :, :],
                                      in_=a2a_out_r[half])

                def proj_units(half, rt):
                    # out rows [rt*128,(rt+1)*128) of owned half: one unit
                    # per (nt, fc-quad) so the PE-queue insertions stay
                    # small while attention runs; the two 512-col output
                    # tiles run sequentially through one psum bank
                    o_sb = misc_pool.tile([128, E], BF16, tag="o",
                                          name=f"o{half}{rt}")
                    for nt in range(E // LT):
                        ps = ps_pool.tile([128, LT], F32, tag="pj", bufs=1,
                                          name=f"proj{half}{rt}{nt}")

                        def fc_run(nt, ps, f0, f1):
                            for fc in range(f0, f1):
                                nc.tensor.matmul(
                                    ps[:],
                                    lhsT=ya_sb[:, half, fc,
                                               rt * 128:(rt + 1) * 128],
                                    rhs=wproj_sb[:, fc,
                                                 nt * LT:(nt + 1) * LT],
                                    start=(fc == 0),
                                    stop=(fc == ECH - 1),
                                )
                            if f1 == ECH:
                                nc.vector.tensor_copy(
                                    out=o_sb[:, nt * LT:(nt + 1) * LT],
                                    in_=ps[:])
                        for f0 in range(0, ECH, 2):
                            yield (lambda nt=nt, ps=ps, f0=f0:
                                   fc_run(nt, ps, f0, f0 + 2))

                    def tail():
                        nc.sync.dma_start(
                            out=out[half, rt * 128:(rt + 1) * 128, :],
                            in_=o_sb[:])
                    yield tail

                def attn_group(qt, pending=(), pending_late=()):
                    # pending: unit closures interleaved evenly across this
                    # group's k-tiles; pending_late only from the midpoint on
                    # (their inputs arrive mid-group).
                    pending = list(pending)
                    late = list(pending_late)
                    nkt = 4 * qt + 4
                    yps = [psy_pool.tile([HD + 1, LT], F32, tag="y",
                                         name=f"yps{qt}_{h}")
                           for h in range(HPC)]
                    s_next = emit_scores(qt, 0)
                    for kt in range(nkt):
                        s_cur = s_next
                        if kt + 1 < nkt:
                            s_next = emit_scores(qt, kt + 1)
                        emit_kt(qt, kt, 0, nkt - 1, yps, s_cur)
                        if kt >= nkt // 2 and late:
                            pending += late
                            late = []
                        n = -(-len(pending) // (nkt - kt))  # ceil
                        for _ in range(n):
                            pending.pop(0)()
                    for u in pending + late:
                        u()
                    emit_qt_tail(qt, yps)

                # startup: first l-tile runs straight (nothing to overlap)
                for u in lt_units(0):
                    u()
                attn_group(0, lt_units(1))
                if repeat is None:
                    nc.sync.dma_start(out=wproj_sb[:], in_=wprojT_r)
                attn_group(1, lt_units(2))
                attn_group(2, lt_units(3))
                attn_group(3, lt_units(4))
                emit_exchange(0)
                attn_group(4, lt_units(5), proj_units(0, 0))
                attn_group(5, lt_units(6), proj_units(0, 1))
                attn_group(6, lt_units(7))
                attn_group(7)
                emit_exchange(1)
                for u in proj_units(1, 0):
                    u()
                for u in proj_units(1, 1):
                    u()

            if repeat is not None:
                with tc.For_i(0, repeat, 1):
                    emit_all()
            else:
                emit_all()

    nc.compile()
    return nc


def shard_inputs(x, w_attn, w_proj):
    """Host-side prep: transpose + bf16 cast + per-core head shards."""
    bf16 = ml_dtypes.bfloat16
    x = np.asarray(x, dtype=np.float32)
    w_attn = np.asarray(w_attn, dtype=np.float32)
    w_proj = np.asarray(w_proj, dtype=np.float32)
    xT = np.ascontiguousarray(x.reshape(L, E).T).astype(bf16)      # [E, L]
    wprojT = np.ascontiguousarray(w_proj.T).astype(bf16)           # [E, E]
    wq, wk, wv = w_attn[0:E], w_attn[E:2 * E], w_attn[2 * E:3 * E]
    in_maps = []
    for c in range(NCORES):
        h0, h1 = HPC * c, HPC * c + 1
        cols = np.concatenate([
            wq[h0 * HD:(h0 + 1) * HD], wq[h1 * HD:(h1 + 1) * HD],
            wk[h0 * HD:(h0 + 1) * HD], wk[h1 * HD:(h1 + 1) * HD],
            wv[h0 * HD:(h0 + 1) * HD], wv[h1 * HD:(h1 + 1) * HD],
        ], axis=0)                                                 # [384, E]
        in_maps.append({
            "xT": xT,
            "wqkvT": np.ascontiguousarray(cols.T).astype(bf16),    # [E, 384]
            "wprojT": wprojT,
        })
    return in_maps


def unshard(outs):
    """outs[c] is core c's [2, 256, E] slab; core c owns queries
    [c*256,(c+1)*256) of each 2048-query half."""
    big = np.empty((2, NCORES, QOWN, E), dtype=np.float32)
    for c in range(NCORES):
        big[:, c] = np.asarray(outs[c], dtype=np.float32).reshape(2, QOWN, E)
    return big.reshape(L, E)


_NC_CACHE = None


def get_program():
    global _NC_CACHE
    if _NC_CACHE is None:
        _NC_CACHE = build_program()
    return _NC_CACHE


def kernel(x, w_attn, w_proj):
    nc = get_program()
    in_maps = shard_inputs(x, w_attn, w_proj)
    res = run_bass_kernel_spmd(nc, in_maps, list(range(NCORES)))
    out = unshard([res.results[c]["out"] for c in range(NCORES)])
    return out.reshape(1, L, E).astype(np.float32)
